# revision 23
# baseline (speedup 1.0000x reference)
"""EngramGating Trainium2 Bass kernel (fp16 pipeline, per-block tails).

Reference computation (per token t, head h, DIM=32, HC_MULT=4):
    key[t,h,:]  = emb[t,:] @ Wk[h].T + bk[h]
    nk = key * rsqrt(mean_k(key^2)+eps) * g1
    nq = hid  * rsqrt(mean_k(hid^2)+eps) * g2
    gate0[t,h] = sum_k nk*nq / sqrt(32)
    ga = sign(gate0)*sqrt(max(|gate0|,1e-6));  gate = sigmoid(ga)
    out[t,h,:] = gate[t,h] * (emb[t,:] @ Wv.T + bv)

Sharding: pure data parallel over 8 cores, contiguous token ranges.

Design (per core, tokens-on-partitions, tpp=18 tokens per partition per
block, 14 full blocks + 1 short):
 - hid arrives fp16 in DRAM; emb arrives HOST-PRE-TRANSPOSED as fp16
   stationary tiles (embt) whose rows 96:128 are kept all-ones on
   device (bias row trick); out is fp16 in DRAM (host converts back).
   fp16 is required: bf16's 8-bit mantissa gives dot errors ~0.05 that
   the sqrt at gate0~0 amplifies past the 2e-2 gate (validated
   numerically; fp16 lands at ~1e-2).
 - One K=128 fp16 matmul per chunk (1 cyc/row) against a block-diagonal
   [Wk|Wv]+bias-row constant produces key|val with biases in PSUM.
 - ACT evacuates PSUM->SBUF fp16 in one merged copy per pair.
 - Squares key^2/hid^2 split ACT(Square)/DVE(tensor_tensor, 2x fp16);
   key*hid on DVE/Pool; sum over k=32 as a 5-level pairwise add tree
   (2x fp16), rows split DVE/Pool.
 - Tail per block (ACT ops all live in one act table - no table
   switches): with S_k=sum key^2, S_q=sum hid^2, d=dot:
     t = sqrt(32)*|d| / sqrt(S_k*S_q) = |gate0|;  |z| = sqrt(t)
     gate = 0.5 + sign(d)*|z|*poly(t),  poly = minimax cubic of
     (sigmoid(z)-0.5)/z on |z| <= 32^(1/4) (Cauchy-Schwarz bound).
 - Final out = gate*val via broadcast tensor_tensor (val read straight
   from the evac tile), rows split Pool/DVE; fp16 DMA out.
"""

import math
import numpy as np
from contextlib import ExitStack

import concourse.bass as bass
import concourse.bacc as bacc
import concourse.mybir as mybir
import concourse.tile as tile
from concourse.bass_utils import run_bass_kernel_spmd

F32 = mybir.dt.float32
F16 = mybir.dt.float16
AF = mybir.ActivationFunctionType
ALU = mybir.AluOpType
AX = mybir.AxisListType

# problem dims
B, S, DIM, H = 16, 16384, 32, 4
TOK = B * S                  # 262144
NCORES = 8
TPC = TOK // NCORES          # 32768 tokens per core
HK = H * DIM                 # 128

# block geometry: 10 full blocks (tpp=24) + 1 short (tpp=18) covering
# the [TPC-2304, TPC) remainder (256-token overlap, rewritten
# idempotently).
TPP = 24
BLK = 128 * TPP
T0S = [i * BLK for i in range(TPC // BLK)] + [TPC - 128 * 18]
TPPS = [TPP] * (TPC // BLK) + [18]
NBLK = len(T0S)
EPS = float(np.finfo(np.float32).eps)

# sigmoid odd-poly: sigmoid(z) ~= 0.5 + z*(c0+c1 t+c2 t^2+c3 t^3),
# t=z^2, max abs err 6e-5 on |z|<=2.4
SC0, SC1, SC2, SC3 = (2.49764353e-01, -2.02204249e-02,
                      1.63422342e-03, -7.25322973e-05)
SQRT32 = math.sqrt(32.0)

# engine split tuning (rows of tpp assigned to the named engine)
SQK_ACT = 21                 # key^2 rows on ACT (rest DVE)
SQQ_ACT = 20                 # hid^2 rows on ACT (rest DVE)
PROD_POOL = 0                # key*hid rows on Pool (rest DVE)
TREE_POOL = 4                # tree rows on Pool (rest DVE)
FIN_DVE = 5                  # final rows on DVE (rest Pool)
EVAC_DVE_SLOTS = ()              # which 2-pair evac slots go to DVE
DMA_AHEAD = 3


def _build_nc(apply_g12: bool, reps: int = 1):
    nc = bacc.Bacc(None, target_bir_lowering=False, debug=False)

    embt_d = nc.dram_tensor("embt", [NBLK * 96 * 1024], F16,
                            kind="ExternalInput")
    hid_d = nc.dram_tensor("hid", [TPC * HK], F16, kind="ExternalInput")
    wkv_d = nc.dram_tensor("wkv", [128, 480], F16, kind="ExternalInput")
    g12_d = None
    if apply_g12:
        g12_d = nc.dram_tensor("g12", [128, HK], F16, kind="ExternalInput")
    out_d = nc.dram_tensor("out", [TPC * HK], F16, kind="ExternalOutput")

    with tile.TileContext(nc) as tc, ExitStack() as ctx:
        const_p = ctx.enter_context(tc.tile_pool(name="const", bufs=1))
        hid_p = ctx.enter_context(tc.tile_pool(name="hidp", bufs=5))
        kvp_p = ctx.enter_context(
            tc.tile_pool(name="kvpp", bufs=3, space=bass.MemorySpace.PSUM))
        kv_p = ctx.enter_context(tc.tile_pool(name="kvp", bufs=3))
        sq_p = ctx.enter_context(tc.tile_pool(name="sqp", bufs=3))
        tr_p = ctx.enter_context(tc.tile_pool(name="trp", bufs=3))
        tail_p = ctx.enter_context(tc.tile_pool(name="tailp", bufs=3))
        out_p = ctx.enter_context(tc.tile_pool(name="outp", bufs=3))

        wkv_sb = const_p.tile([128, 480], F16)
        nc.sync.dma_start(wkv_sb[:], wkv_d[:])
        if apply_g12:
            g12_sb = const_p.tile([128, HK], F16)
            nc.sync.dma_start(g12_sb[:], g12_d[:])

        embt_tiles = []
        for i in range(DMA_AHEAD + 1):
            t = const_p.tile([128, 4, 2, 128], F16, name=f"embt{i}")
            nc.gpsimd.memset(t[96:128, :, :, :], 1.0)
            embt_tiles.append(t)
        NEMBT = len(embt_tiles)

        def stage_dma(b, idx):
            # input DMA issue, DMA_AHEAD blocks ahead of use.  embt rows
            # 0:96 come host-pre-transposed from DRAM; rows 96:128 stay
            # all-ones (bias rows, memset once at startup).
            t0 = T0S[b]
            tpp = TPPS[b]
            blk = 128 * tpp
            npair = tpp // 6
            embt = embt_tiles[idx % NEMBT]
            nc.sync.dma_start(
                embt[0:96, 0:npair, :, :].rearrange("p a b c -> p (a b c)"),
                embt_d[b * 96 * 1024:(b + 1) * 96 * 1024].rearrange(
                    "(p f) -> p f", p=96)[:, 0:npair * 256])
            hid_sb = hid_p.tile([128, tpp, H, DIM], F16, name="hid_sb")
            nc.sync.dma_start(
                hid_sb[:].rearrange("p a b c -> p (a b c)"),
                hid_d[t0 * HK:(t0 + blk) * HK].rearrange(
                    "(p f) -> p f", p=128))
            return embt, hid_sb

        def emit_block(b, staged, fd_rows=None):
            t0 = T0S[b]
            tpp = TPPS[b]
            blk = 128 * tpp
            npair = tpp // 6
            embt, hid_sb = staged

            # kv matmuls (fp16, 1 cyc/row) + merged ACT evac per pair
            kv_sb = kv_p.tile([128, 8, 3, 160], F16, name="kv_sb")
            for g in range(npair):
                kvp = kvp_p.tile([128, 2, 512], F32, name="kvp")
                for c2 in range(2):
                    nc.tensor.matmul(
                        kvp[:, c2, 0:480],
                        embt[:, g, c2, :],
                        wkv_sb[:, 0:480],
                        start=True, stop=True)
                nc.scalar.copy(
                    kv_sb[:, 2 * g:2 * (g + 1), :, :],
                    kvp[:, :, 0:480].rearrange("p c (j m) -> p c j m", m=160))

            key4 = kv_sb[:, 0:2 * npair, :, 0:HK].rearrange(
                "p a b (h k) -> p (a b) h k", h=H)     # [128, tpp, H, K]
            val3 = kv_sb[:, 0:2 * npair, :, HK:160].rearrange(
                "p a b k -> p (a b) k")                # [128, tpp, K]

            if apply_g12:
                hidg = sq_p.tile([128, TPP, H, DIM], F16, name="hidg")
                nc.vector.tensor_tensor(
                    hidg[:, 0:tpp], hid_sb[:],
                    g12_sb[:].rearrange("p (o h k) -> p o h k", o=1, h=H)
                    .broadcast_to([128, tpp, H, DIM]),
                    op=ALU.mult)
                hid4 = hidg[:, 0:tpp]
            else:
                hid4 = hid_sb[:]

            # squares + product into one [128, 3, tpp, H, K] tile
            sq3 = sq_p.tile([128, 3, TPP, H, DIM], F16, name="sq3")
            ka = min(SQK_ACT, tpp)
            if ka > 0:
                nc.scalar.activation(sq3[:, 0, 0:ka], key4[:, 0:ka],
                                     AF.Square)
            if ka < tpp:
                nc.vector.tensor_tensor(sq3[:, 0, ka:tpp], key4[:, ka:tpp],
                                        key4[:, ka:tpp], op=ALU.mult)
            qa = min(SQQ_ACT, tpp)
            if qa > 0:
                nc.scalar.activation(sq3[:, 1, 0:qa], hid4[:, 0:qa],
                                     AF.Square)
            if qa < tpp:
                nc.vector.tensor_tensor(sq3[:, 1, qa:tpp], hid4[:, qa:tpp],
                                        hid4[:, qa:tpp], op=ALU.mult)
            pp = min(PROD_POOL, tpp)
            if pp > 0:
                nc.gpsimd.tensor_tensor(sq3[:, 2, 0:pp], key4[:, 0:pp],
                                        hid4[:, 0:pp], op=ALU.mult)
            if pp < tpp:
                nc.vector.tensor_tensor(sq3[:, 2, pp:tpp], key4[:, pp:tpp],
                                        hid4[:, pp:tpp], op=ALU.mult)

            # 5-level pairwise add tree over k (fp16, 2x), rows split
            # DVE/Pool; L5 writes fp32 stats.
            trA = tr_p.tile([128, 3, TPP, H, 16], F16, name="trA")
            trB = tr_p.tile([128, 3, TPP, H, 8], F16, name="trB")
            stats = tr_p.tile([128, 3, TPP, H], F32, name="stats")
            tpool = min(TREE_POOL, tpp)

            def level(dst, a, b_):
                if tpool > 0:
                    nc.gpsimd.tensor_tensor(
                        dst[:, :, 0:tpool], a[:, :, 0:tpool],
                        b_[:, :, 0:tpool], op=ALU.add)
                if tpool < tpp:
                    nc.vector.tensor_tensor(
                        dst[:, :, tpool:tpp], a[:, :, tpool:tpp],
                        b_[:, :, tpool:tpp], op=ALU.add)

            s3 = sq3[:, :, 0:tpp]
            level(trA[:, :, 0:tpp], s3[:, :, :, :, 0:16],
                  s3[:, :, :, :, 16:32])
            a16 = trA[:, :, 0:tpp]
            level(trB[:, :, 0:tpp], a16[:, :, :, :, 0:8],
                  a16[:, :, :, :, 8:16])
            b8 = trB[:, :, 0:tpp]
            level(trA[:, :, 0:tpp, :, 0:4], b8[:, :, :, :, 0:4],
                  b8[:, :, :, :, 4:8])
            a4 = trA[:, :, 0:tpp, :, 0:4]
            level(trB[:, :, 0:tpp, :, 0:2], a4[:, :, :, :, 0:2],
                  a4[:, :, :, :, 2:4])
            b2 = trB[:, :, 0:tpp, :, 0:2]
            level(stats[:, :, 0:tpp].unsqueeze(4),
                  b2[:, :, :, :, 0:1], b2[:, :, :, :, 1:2])

            # ---- per-block tail ----
            FT = tpp * H
            msk = stats[:, 0, 0:tpp].rearrange("p a b -> p (a b)")
            msq = stats[:, 1, 0:tpp].rearrange("p a b -> p (a b)")
            dot = stats[:, 2, 0:tpp].rearrange("p a b -> p (a b)")

            P = tail_p.tile([128, FT], F32, name="P", tag="P")
            nc.vector.tensor_tensor(P[:], msk, msq, op=ALU.mult)
            sP = tail_p.tile([128, FT], F32, name="sP", tag="sP")
            nc.scalar.activation(sP[:], P[:], AF.Sqrt)
            w = tail_p.tile([128, FT], F32, name="w", tag="w")
            nc.vector.reciprocal(w[:], sP[:])
            ad = tail_p.tile([128, FT], F32, name="ad", tag="ad")
            nc.scalar.activation(ad[:], dot, AF.Abs, scale=SQRT32)
            t = tail_p.tile([128, FT], F32, name="t", tag="t")
            nc.vector.tensor_tensor(t[:], ad[:], w[:], op=ALU.mult)
            tr = tail_p.tile([128, FT], F32, name="tr", tag="tr")
            nc.scalar.activation(tr[:], t[:], AF.Sqrt)
            sg = tail_p.tile([128, FT], F32, name="sg", tag="sg")
            nc.scalar.activation(sg[:], dot, AF.Sign)
            h = tail_p.tile([128, FT], F32, name="h", tag="h")
            nc.vector.tensor_scalar(h[:], t[:], SC3, SC2,
                                    op0=ALU.mult, op1=ALU.add)
            h2 = tail_p.tile([128, FT], F32, name="h2", tag="h2")
            nc.vector.tensor_tensor(h2[:], h[:], t[:], op=ALU.mult)
            nc.vector.tensor_scalar(h2[:], h2[:], SC1, None, op0=ALU.add)
            nc.vector.tensor_tensor(h2[:], h2[:], t[:], op=ALU.mult)
            nc.vector.tensor_scalar(h2[:], h2[:], SC0, None, op0=ALU.add)
            nc.vector.tensor_tensor(h2[:], h2[:], tr[:], op=ALU.mult)
            nc.vector.tensor_tensor(h2[:], h2[:], sg[:], op=ALU.mult)
            gate = tail_p.tile([128, TPP, H], F16, name="gate")
            nc.vector.tensor_scalar(
                gate[:, 0:tpp].rearrange("p a b -> p (a b)"),
                h2[:], 0.5, None, op0=ALU.add)

            # ---- final gating + store ----
            out_sb = out_p.tile([128, TPP, H, DIM], F16, name="out_sb")
            gate_b = gate[:, 0:tpp, :].unsqueeze(3)
            val_b = val3.unsqueeze(2)
            fd = min(FIN_DVE if fd_rows is None else fd_rows, tpp)
            if fd > 0:
                nc.vector.tensor_tensor(
                    out_sb[:, 0:fd],
                    gate_b[:, 0:fd].broadcast_to([128, fd, H, DIM]),
                    val_b[:, 0:fd].broadcast_to([128, fd, H, DIM]),
                    op=ALU.mult)
            if fd < tpp:
                nc.gpsimd.tensor_tensor(
                    out_sb[:, fd:tpp],
                    gate_b[:, fd:tpp].broadcast_to([128, tpp - fd, H, DIM]),
                    val_b[:, fd:tpp].broadcast_to([128, tpp - fd, H, DIM]),
                    op=ALU.mult)
            nc.sync.dma_start(
                out_d[t0 * HK:(t0 + blk) * HK].rearrange(
                    "(p f) -> p f", p=128),
                out_sb[:, 0:tpp].rearrange("p a b c -> p (a b c)"))

        blocks = [b for _ in range(reps) for b in range(NBLK)]
        dmas = {}
        for j in range(min(DMA_AHEAD, len(blocks))):
            dmas[j] = stage_dma(blocks[j], j)
        for i, b in enumerate(blocks):
            if i + DMA_AHEAD < len(blocks):
                dmas[i + DMA_AHEAD] = stage_dma(blocks[i + DMA_AHEAD],
                                                i + DMA_AHEAD)
            # the last two blocks' finals split evenly DVE/Pool so the
            # end-of-kernel drain isn't serialized on Pool
            fd_rows = TPPS[b] // 2 if i >= len(blocks) - 2 else None
            emit_block(b, dmas.pop(i), fd_rows=fd_rows)

    nc.compile()
    return nc


def _prep_embt(emb_flat_f16):
    # embt[b, s*32+d, cc, p] = emb[t0 + p*tpp + 3*cc + s, d]; rows
    # 96:128 (the ones bias rows) live on-device, not in DRAM.
    out = np.zeros((NBLK, 96, 8, 128), dtype=np.float16)
    for b, (t0, tpp) in enumerate(zip(T0S, TPPS)):
        blk = 128 * tpp
        E = emb_flat_f16[t0:t0 + blk].reshape(128, 2 * (tpp // 6), 3, DIM)
        out[b, :, 0:2 * (tpp // 6), :] = np.transpose(
            E, (2, 3, 1, 0)).reshape(96, 2 * (tpp // 6), 128)
    return np.ascontiguousarray(out.reshape(-1))


def _prep_consts(Wv, bv, Wk, bk):
    # Wkv_cat[d, h*32+k] = Wk[h,k,d];  Wkv_cat[d, 128+v] = Wv[v,d]
    wkv_cat = np.zeros((DIM, 160), dtype=np.float32)
    wkv_cat[:, 0:HK] = np.transpose(Wk, (2, 0, 1)).reshape(DIM, HK)
    wkv_cat[:, HK:160] = Wv.T
    bias_cat = np.concatenate(
        [bk.reshape(HK).astype(np.float32), bv.astype(np.float32)])
    wkv = np.zeros((128, 480), dtype=np.float32)
    for j in range(3):
        wkv[32 * j:32 * (j + 1), 160 * j:160 * (j + 1)] = wkv_cat
    wkv[96, :] = np.tile(bias_cat, 3)
    return wkv.astype(np.float16)


_CACHE = {}


def kernel_with_results(embeddings, hidden_states, Wv, bv, Wk, bk, g1, g2,
                        **run_kwargs):
    embeddings = np.asarray(embeddings, dtype=np.float32)
    hidden_states = np.asarray(hidden_states, dtype=np.float32)
    Wv = np.asarray(Wv, dtype=np.float32)
    bv = np.asarray(bv, dtype=np.float32)
    Wk = np.asarray(Wk, dtype=np.float32)
    bk = np.asarray(bk, dtype=np.float32)
    g12 = (np.asarray(g1, np.float32) * np.asarray(g2, np.float32))
    apply_g12 = not np.all(g12 == 1.0)

    if apply_g12 not in _CACHE:
        _CACHE[apply_g12] = _build_nc(apply_g12)
    nc = _CACHE[apply_g12]

    wkv = _prep_consts(Wv, bv, Wk, bk)

    emb_flat = np.ascontiguousarray(
        embeddings.reshape(TOK, DIM).astype(np.float16))
    hid_flat = np.ascontiguousarray(
        hidden_states.reshape(TOK, HK).astype(np.float16))

    in_maps = []
    for c in range(NCORES):
        m = {
            "embt": _prep_embt(emb_flat[c * TPC:(c + 1) * TPC]),
            "hid": np.ascontiguousarray(
                hid_flat[c * TPC:(c + 1) * TPC]).reshape(-1),
            "wkv": wkv,
        }
        if apply_g12:
            m["g12"] = np.tile(
                g12.reshape(1, HK), (128, 1)).astype(np.float16)
        in_maps.append(m)

    res = run_bass_kernel_spmd(nc, in_maps, core_ids=list(range(NCORES)),
                               **run_kwargs)
    out = np.concatenate(
        [np.asarray(res.results[c]["out"]).reshape(TPC, HK)
         for c in range(NCORES)],
        axis=0)
    return out.astype(np.float32).reshape(B, S, H, DIM), res


def kernel(embeddings, hidden_states, Wv, bv, Wk, bk, g1, g2):
    out, _ = kernel_with_results(
        embeddings, hidden_states, Wv, bv, Wk, bk, g1, g2)
    return out


# revision 24
# speedup vs baseline: 1.0277x; 1.0277x over previous
"""EngramGating Trainium2 Bass kernel (fp16 pipeline, per-block tails).

Reference computation (per token t, head h, DIM=32, HC_MULT=4):
    key[t,h,:]  = emb[t,:] @ Wk[h].T + bk[h]
    nk = key * rsqrt(mean_k(key^2)+eps) * g1
    nq = hid  * rsqrt(mean_k(hid^2)+eps) * g2
    gate0[t,h] = sum_k nk*nq / sqrt(32)
    ga = sign(gate0)*sqrt(max(|gate0|,1e-6));  gate = sigmoid(ga)
    out[t,h,:] = gate[t,h] * (emb[t,:] @ Wv.T + bv)

Sharding: pure data parallel over 8 cores, contiguous token ranges.

Design (per core, tokens-on-partitions, tpp=18 tokens per partition per
block, 14 full blocks + 1 short):
 - hid arrives fp16 in DRAM; emb arrives HOST-PRE-TRANSPOSED as fp16
   stationary tiles (embt) whose rows 96:128 are kept all-ones on
   device (bias row trick); out is fp16 in DRAM (host converts back).
   fp16 is required: bf16's 8-bit mantissa gives dot errors ~0.05 that
   the sqrt at gate0~0 amplifies past the 2e-2 gate (validated
   numerically; fp16 lands at ~1e-2).
 - One K=128 fp16 matmul per chunk (1 cyc/row) against a block-diagonal
   [Wk|Wv]+bias-row constant produces key|val with biases in PSUM.
 - ACT evacuates PSUM->SBUF fp16 in one merged copy per pair.
 - Squares key^2/hid^2 split ACT(Square)/DVE(tensor_tensor, 2x fp16);
   key*hid on DVE/Pool; sum over k=32 as a 5-level pairwise add tree
   (2x fp16), rows split DVE/Pool.
 - Tail per block (ACT ops all live in one act table - no table
   switches): with S_k=sum key^2, S_q=sum hid^2, d=dot:
     t = sqrt(32)*|d| / sqrt(S_k*S_q) = |gate0|;  |z| = sqrt(t)
     gate = 0.5 + sign(d)*|z|*poly(t),  poly = minimax cubic of
     (sigmoid(z)-0.5)/z on |z| <= 32^(1/4) (Cauchy-Schwarz bound).
 - Final out = gate*val via broadcast tensor_tensor (val read straight
   from the evac tile), rows split Pool/DVE; fp16 DMA out.
"""

import math
import numpy as np
from contextlib import ExitStack

import concourse.bass as bass
import concourse.bacc as bacc
import concourse.mybir as mybir
import concourse.tile as tile
from concourse.bass_utils import run_bass_kernel_spmd

F32 = mybir.dt.float32
F16 = mybir.dt.float16
AF = mybir.ActivationFunctionType
ALU = mybir.AluOpType
AX = mybir.AxisListType

# problem dims
B, S, DIM, H = 16, 16384, 32, 4
TOK = B * S                  # 262144
NCORES = 8
TPC = TOK // NCORES          # 32768 tokens per core
HK = H * DIM                 # 128

# block geometry: 10 full blocks (tpp=24) + 1 short (tpp=18) covering
# the [TPC-2304, TPC) remainder (256-token overlap, rewritten
# idempotently).
TPP = 24
BLK = 128 * TPP
T0S = [i * BLK for i in range(TPC // BLK)] + [TPC - 128 * 18]
TPPS = [TPP] * (TPC // BLK) + [18]
NBLK = len(T0S)
EPS = float(np.finfo(np.float32).eps)

# sigmoid odd-poly: sigmoid(z) ~= 0.5 + z*(c0+c1 t+c2 t^2+c3 t^3),
# t=z^2, max abs err 6e-5 on |z|<=2.4
SC0, SC1, SC2, SC3 = (2.49764353e-01, -2.02204249e-02,
                      1.63422342e-03, -7.25322973e-05)
SQRT32 = math.sqrt(32.0)

# engine split tuning (rows of tpp assigned to the named engine)
SQK_ACT = 19                 # key^2 rows on ACT (rest DVE)
SQQ_ACT = 19                 # hid^2 rows on ACT (rest DVE)
PROD_POOL = 2                # key*hid rows on Pool (rest DVE)
TREE_POOL = 4                # tree rows on Pool (rest DVE)
FIN_DVE = 8                  # final rows on DVE (rest Pool)
EVAC_DVE_SLOTS = ()              # which 2-pair evac slots go to DVE
DMA_AHEAD = 3


def _build_nc(apply_g12: bool, reps: int = 1):
    nc = bacc.Bacc(None, target_bir_lowering=False, debug=False)

    embt_d = nc.dram_tensor("embt", [NBLK * 96 * 1024], F16,
                            kind="ExternalInput")
    hid_d = nc.dram_tensor("hid", [TPC * HK], F16, kind="ExternalInput")
    wkv_d = nc.dram_tensor("wkv", [128, 480], F16, kind="ExternalInput")
    g12_d = None
    if apply_g12:
        g12_d = nc.dram_tensor("g12", [128, HK], F16, kind="ExternalInput")
    out_d = nc.dram_tensor("out", [TPC * HK], F16, kind="ExternalOutput")

    with tile.TileContext(nc) as tc, ExitStack() as ctx:
        const_p = ctx.enter_context(tc.tile_pool(name="const", bufs=1))
        hid_p = ctx.enter_context(tc.tile_pool(name="hidp", bufs=5))
        kvp_p = ctx.enter_context(
            tc.tile_pool(name="kvpp", bufs=3, space=bass.MemorySpace.PSUM))
        kv_p = ctx.enter_context(tc.tile_pool(name="kvp", bufs=3))
        sq_p = ctx.enter_context(tc.tile_pool(name="sqp", bufs=3))
        tr_p = ctx.enter_context(tc.tile_pool(name="trp", bufs=3))
        tail_p = ctx.enter_context(tc.tile_pool(name="tailp", bufs=3))
        out_p = ctx.enter_context(tc.tile_pool(name="outp", bufs=3))

        wkv_sb = const_p.tile([128, 480], F16)
        nc.sync.dma_start(wkv_sb[:], wkv_d[:])
        if apply_g12:
            g12_sb = const_p.tile([128, HK], F16)
            nc.sync.dma_start(g12_sb[:], g12_d[:])

        embt_tiles = []
        for i in range(DMA_AHEAD + 1):
            t = const_p.tile([128, 4, 2, 128], F16, name=f"embt{i}")
            nc.gpsimd.memset(t[96:128, :, :, :], 1.0)
            embt_tiles.append(t)
        NEMBT = len(embt_tiles)

        def stage_dma(b, idx):
            # input DMA issue, DMA_AHEAD blocks ahead of use.  embt rows
            # 0:96 come host-pre-transposed from DRAM; rows 96:128 stay
            # all-ones (bias rows, memset once at startup).
            t0 = T0S[b]
            tpp = TPPS[b]
            blk = 128 * tpp
            npair = tpp // 6
            embt = embt_tiles[idx % NEMBT]
            nc.sync.dma_start(
                embt[0:96, 0:npair, :, :].rearrange("p a b c -> p (a b c)"),
                embt_d[b * 96 * 1024:(b + 1) * 96 * 1024].rearrange(
                    "(p f) -> p f", p=96)[:, 0:npair * 256])
            hid_sb = hid_p.tile([128, tpp, H, DIM], F16, name="hid_sb")
            nc.sync.dma_start(
                hid_sb[:].rearrange("p a b c -> p (a b c)"),
                hid_d[t0 * HK:(t0 + blk) * HK].rearrange(
                    "(p f) -> p f", p=128))
            return embt, hid_sb

        def emit_block(b, staged, fd_rows=None):
            t0 = T0S[b]
            tpp = TPPS[b]
            blk = 128 * tpp
            npair = tpp // 6
            embt, hid_sb = staged

            # kv matmuls (fp16, 1 cyc/row) + merged ACT evac per pair
            kv_sb = kv_p.tile([128, 8, 3, 160], F16, name="kv_sb")
            for g in range(npair):
                kvp = kvp_p.tile([128, 2, 512], F32, name="kvp")
                for c2 in range(2):
                    nc.tensor.matmul(
                        kvp[:, c2, 0:480],
                        embt[:, g, c2, :],
                        wkv_sb[:, 0:480],
                        start=True, stop=True)
                nc.scalar.copy(
                    kv_sb[:, 2 * g:2 * (g + 1), :, :],
                    kvp[:, :, 0:480].rearrange("p c (j m) -> p c j m", m=160))

            key4 = kv_sb[:, 0:2 * npair, :, 0:HK].rearrange(
                "p a b (h k) -> p (a b) h k", h=H)     # [128, tpp, H, K]
            val3 = kv_sb[:, 0:2 * npair, :, HK:160].rearrange(
                "p a b k -> p (a b) k")                # [128, tpp, K]

            if apply_g12:
                hidg = sq_p.tile([128, TPP, H, DIM], F16, name="hidg")
                nc.vector.tensor_tensor(
                    hidg[:, 0:tpp], hid_sb[:],
                    g12_sb[:].rearrange("p (o h k) -> p o h k", o=1, h=H)
                    .broadcast_to([128, tpp, H, DIM]),
                    op=ALU.mult)
                hid4 = hidg[:, 0:tpp]
            else:
                hid4 = hid_sb[:]

            # squares + product into one [128, 3, tpp, H, K] tile
            sq3 = sq_p.tile([128, 3, TPP, H, DIM], F16, name="sq3")
            ka = min(SQK_ACT, tpp)
            if ka > 0:
                nc.scalar.activation(sq3[:, 0, 0:ka], key4[:, 0:ka],
                                     AF.Square)
            if ka < tpp:
                nc.vector.tensor_tensor(sq3[:, 0, ka:tpp], key4[:, ka:tpp],
                                        key4[:, ka:tpp], op=ALU.mult)
            qa = min(SQQ_ACT, tpp)
            if qa > 0:
                nc.scalar.activation(sq3[:, 1, 0:qa], hid4[:, 0:qa],
                                     AF.Square)
            if qa < tpp:
                nc.vector.tensor_tensor(sq3[:, 1, qa:tpp], hid4[:, qa:tpp],
                                        hid4[:, qa:tpp], op=ALU.mult)
            pp = min(PROD_POOL, tpp)
            if pp > 0:
                nc.gpsimd.tensor_tensor(sq3[:, 2, 0:pp], key4[:, 0:pp],
                                        hid4[:, 0:pp], op=ALU.mult)
            if pp < tpp:
                nc.vector.tensor_tensor(sq3[:, 2, pp:tpp], key4[:, pp:tpp],
                                        hid4[:, pp:tpp], op=ALU.mult)

            # 5-level pairwise add tree over k (fp16, 2x), rows split
            # DVE/Pool; L5 writes fp32 stats.
            trA = tr_p.tile([128, 3, TPP, H, 16], F16, name="trA")
            trB = tr_p.tile([128, 3, TPP, H, 8], F16, name="trB")
            stats = tr_p.tile([128, 3, TPP, H], F32, name="stats")
            tpool = min(TREE_POOL, tpp)

            def level(dst, a, b_):
                if tpool > 0:
                    nc.gpsimd.tensor_tensor(
                        dst[:, :, 0:tpool], a[:, :, 0:tpool],
                        b_[:, :, 0:tpool], op=ALU.add)
                if tpool < tpp:
                    nc.vector.tensor_tensor(
                        dst[:, :, tpool:tpp], a[:, :, tpool:tpp],
                        b_[:, :, tpool:tpp], op=ALU.add)

            s3 = sq3[:, :, 0:tpp]
            level(trA[:, :, 0:tpp], s3[:, :, :, :, 0:16],
                  s3[:, :, :, :, 16:32])
            a16 = trA[:, :, 0:tpp]
            level(trB[:, :, 0:tpp], a16[:, :, :, :, 0:8],
                  a16[:, :, :, :, 8:16])
            b8 = trB[:, :, 0:tpp]
            level(trA[:, :, 0:tpp, :, 0:4], b8[:, :, :, :, 0:4],
                  b8[:, :, :, :, 4:8])
            a4 = trA[:, :, 0:tpp, :, 0:4]
            level(trB[:, :, 0:tpp, :, 0:2], a4[:, :, :, :, 0:2],
                  a4[:, :, :, :, 2:4])
            b2 = trB[:, :, 0:tpp, :, 0:2]
            level(stats[:, :, 0:tpp].unsqueeze(4),
                  b2[:, :, :, :, 0:1], b2[:, :, :, :, 1:2])

            # ---- per-block tail ----
            FT = tpp * H
            msk = stats[:, 0, 0:tpp].rearrange("p a b -> p (a b)")
            msq = stats[:, 1, 0:tpp].rearrange("p a b -> p (a b)")
            dot = stats[:, 2, 0:tpp].rearrange("p a b -> p (a b)")

            P = tail_p.tile([128, FT], F32, name="P", tag="P")
            nc.vector.tensor_tensor(P[:], msk, msq, op=ALU.mult)
            sP = tail_p.tile([128, FT], F32, name="sP", tag="sP")
            nc.scalar.activation(sP[:], P[:], AF.Sqrt)
            w = tail_p.tile([128, FT], F32, name="w", tag="w")
            nc.vector.reciprocal(w[:], sP[:])
            ad = tail_p.tile([128, FT], F32, name="ad", tag="ad")
            nc.scalar.activation(ad[:], dot, AF.Abs, scale=SQRT32)
            t = tail_p.tile([128, FT], F32, name="t", tag="t")
            nc.vector.tensor_tensor(t[:], ad[:], w[:], op=ALU.mult)
            tr = tail_p.tile([128, FT], F32, name="tr", tag="tr")
            nc.scalar.activation(tr[:], t[:], AF.Sqrt)
            sg = tail_p.tile([128, FT], F32, name="sg", tag="sg")
            nc.scalar.activation(sg[:], dot, AF.Sign)
            h = tail_p.tile([128, FT], F32, name="h", tag="h")
            nc.vector.tensor_scalar(h[:], t[:], SC3, SC2,
                                    op0=ALU.mult, op1=ALU.add)
            h2 = tail_p.tile([128, FT], F32, name="h2", tag="h2")
            nc.vector.tensor_tensor(h2[:], h[:], t[:], op=ALU.mult)
            nc.vector.tensor_scalar(h2[:], h2[:], SC1, None, op0=ALU.add)
            nc.vector.tensor_tensor(h2[:], h2[:], t[:], op=ALU.mult)
            nc.vector.tensor_scalar(h2[:], h2[:], SC0, None, op0=ALU.add)
            nc.vector.tensor_tensor(h2[:], h2[:], tr[:], op=ALU.mult)
            nc.vector.tensor_tensor(h2[:], h2[:], sg[:], op=ALU.mult)
            gate = tail_p.tile([128, TPP, H], F16, name="gate")
            nc.vector.tensor_scalar(
                gate[:, 0:tpp].rearrange("p a b -> p (a b)"),
                h2[:], 0.5, None, op0=ALU.add)

            # ---- final gating + store ----
            out_sb = out_p.tile([128, TPP, H, DIM], F16, name="out_sb")
            gate_b = gate[:, 0:tpp, :].unsqueeze(3)
            val_b = val3.unsqueeze(2)
            fd = min(FIN_DVE if fd_rows is None else fd_rows, tpp)
            if fd > 0:
                nc.vector.tensor_tensor(
                    out_sb[:, 0:fd],
                    gate_b[:, 0:fd].broadcast_to([128, fd, H, DIM]),
                    val_b[:, 0:fd].broadcast_to([128, fd, H, DIM]),
                    op=ALU.mult)
            if fd < tpp:
                nc.gpsimd.tensor_tensor(
                    out_sb[:, fd:tpp],
                    gate_b[:, fd:tpp].broadcast_to([128, tpp - fd, H, DIM]),
                    val_b[:, fd:tpp].broadcast_to([128, tpp - fd, H, DIM]),
                    op=ALU.mult)
            nc.sync.dma_start(
                out_d[t0 * HK:(t0 + blk) * HK].rearrange(
                    "(p f) -> p f", p=128),
                out_sb[:, 0:tpp].rearrange("p a b c -> p (a b c)"))

        blocks = [b for _ in range(reps) for b in range(NBLK)]
        dmas = {}
        for j in range(min(DMA_AHEAD, len(blocks))):
            dmas[j] = stage_dma(blocks[j], j)
        for i, b in enumerate(blocks):
            if i + DMA_AHEAD < len(blocks):
                dmas[i + DMA_AHEAD] = stage_dma(blocks[i + DMA_AHEAD],
                                                i + DMA_AHEAD)
            # the last two blocks' finals split evenly DVE/Pool so the
            # end-of-kernel drain isn't serialized on Pool
            fd_rows = TPPS[b] // 2 if i >= len(blocks) - 2 else None
            emit_block(b, dmas.pop(i), fd_rows=fd_rows)

    nc.compile()
    return nc


def _prep_embt(emb_flat_f16):
    # embt[b, s*32+d, cc, p] = emb[t0 + p*tpp + 3*cc + s, d]; rows
    # 96:128 (the ones bias rows) live on-device, not in DRAM.
    out = np.zeros((NBLK, 96, 8, 128), dtype=np.float16)
    for b, (t0, tpp) in enumerate(zip(T0S, TPPS)):
        blk = 128 * tpp
        E = emb_flat_f16[t0:t0 + blk].reshape(128, 2 * (tpp // 6), 3, DIM)
        out[b, :, 0:2 * (tpp // 6), :] = np.transpose(
            E, (2, 3, 1, 0)).reshape(96, 2 * (tpp // 6), 128)
    return np.ascontiguousarray(out.reshape(-1))


def _prep_consts(Wv, bv, Wk, bk):
    # Wkv_cat[d, h*32+k] = Wk[h,k,d];  Wkv_cat[d, 128+v] = Wv[v,d]
    wkv_cat = np.zeros((DIM, 160), dtype=np.float32)
    wkv_cat[:, 0:HK] = np.transpose(Wk, (2, 0, 1)).reshape(DIM, HK)
    wkv_cat[:, HK:160] = Wv.T
    bias_cat = np.concatenate(
        [bk.reshape(HK).astype(np.float32), bv.astype(np.float32)])
    wkv = np.zeros((128, 480), dtype=np.float32)
    for j in range(3):
        wkv[32 * j:32 * (j + 1), 160 * j:160 * (j + 1)] = wkv_cat
    wkv[96, :] = np.tile(bias_cat, 3)
    return wkv.astype(np.float16)


_CACHE = {}


def kernel_with_results(embeddings, hidden_states, Wv, bv, Wk, bk, g1, g2,
                        **run_kwargs):
    embeddings = np.asarray(embeddings, dtype=np.float32)
    hidden_states = np.asarray(hidden_states, dtype=np.float32)
    Wv = np.asarray(Wv, dtype=np.float32)
    bv = np.asarray(bv, dtype=np.float32)
    Wk = np.asarray(Wk, dtype=np.float32)
    bk = np.asarray(bk, dtype=np.float32)
    g12 = (np.asarray(g1, np.float32) * np.asarray(g2, np.float32))
    apply_g12 = not np.all(g12 == 1.0)

    if apply_g12 not in _CACHE:
        _CACHE[apply_g12] = _build_nc(apply_g12)
    nc = _CACHE[apply_g12]

    wkv = _prep_consts(Wv, bv, Wk, bk)

    emb_flat = np.ascontiguousarray(
        embeddings.reshape(TOK, DIM).astype(np.float16))
    hid_flat = np.ascontiguousarray(
        hidden_states.reshape(TOK, HK).astype(np.float16))

    in_maps = []
    for c in range(NCORES):
        m = {
            "embt": _prep_embt(emb_flat[c * TPC:(c + 1) * TPC]),
            "hid": np.ascontiguousarray(
                hid_flat[c * TPC:(c + 1) * TPC]).reshape(-1),
            "wkv": wkv,
        }
        if apply_g12:
            m["g12"] = np.tile(
                g12.reshape(1, HK), (128, 1)).astype(np.float16)
        in_maps.append(m)

    res = run_bass_kernel_spmd(nc, in_maps, core_ids=list(range(NCORES)),
                               **run_kwargs)
    out = np.concatenate(
        [np.asarray(res.results[c]["out"]).reshape(TPC, HK)
         for c in range(NCORES)],
        axis=0)
    return out.astype(np.float32).reshape(B, S, H, DIM), res


def kernel(embeddings, hidden_states, Wv, bv, Wk, bk, g1, g2):
    out, _ = kernel_with_results(
        embeddings, hidden_states, Wv, bv, Wk, bk, g1, g2)
    return out


# revision 30
# speedup vs baseline: 1.0410x; 1.0129x over previous
"""EngramGating Trainium2 Bass kernel (fp16 pipeline, per-block tails).

Reference computation (per token t, head h, DIM=32, HC_MULT=4):
    key[t,h,:]  = emb[t,:] @ Wk[h].T + bk[h]
    nk = key * rsqrt(mean_k(key^2)+eps) * g1
    nq = hid  * rsqrt(mean_k(hid^2)+eps) * g2
    gate0[t,h] = sum_k nk*nq / sqrt(32)
    ga = sign(gate0)*sqrt(max(|gate0|,1e-6));  gate = sigmoid(ga)
    out[t,h,:] = gate[t,h] * (emb[t,:] @ Wv.T + bv)

Sharding: pure data parallel over 8 cores, contiguous token ranges.

Design (per core, tokens-on-partitions, tpp=18 tokens per partition per
block, 14 full blocks + 1 short):
 - hid arrives fp16 in DRAM; emb arrives HOST-PRE-TRANSPOSED as fp16
   stationary tiles (embt) whose rows 96:128 are kept all-ones on
   device (bias row trick); out is fp16 in DRAM (host converts back).
   fp16 is required: bf16's 8-bit mantissa gives dot errors ~0.05 that
   the sqrt at gate0~0 amplifies past the 2e-2 gate (validated
   numerically; fp16 lands at ~1e-2).
 - One K=128 fp16 matmul per chunk (1 cyc/row) against a block-diagonal
   [Wk|Wv]+bias-row constant produces key|val with biases in PSUM.
 - ACT evacuates PSUM->SBUF fp16 in one merged copy per pair.
 - Squares key^2/hid^2 split ACT(Square)/DVE(tensor_tensor, 2x fp16);
   key*hid on DVE/Pool; sum over k=32 as a 5-level pairwise add tree
   (2x fp16), rows split DVE/Pool.
 - Tail per block (ACT ops all live in one act table - no table
   switches): with S_k=sum key^2, S_q=sum hid^2, d=dot:
     t = sqrt(32)*|d| / sqrt(S_k*S_q) = |gate0|;  |z| = sqrt(t)
     gate = 0.5 + sign(d)*|z|*poly(t),  poly = minimax cubic of
     (sigmoid(z)-0.5)/z on |z| <= 32^(1/4) (Cauchy-Schwarz bound).
 - Final out = gate*val via broadcast tensor_tensor (val read straight
   from the evac tile), rows split Pool/DVE; fp16 DMA out.
"""

import math
import numpy as np
from contextlib import ExitStack

import concourse.bass as bass
import concourse.bacc as bacc
import concourse.mybir as mybir
import concourse.tile as tile
from concourse.bass_utils import run_bass_kernel_spmd

F32 = mybir.dt.float32
F16 = mybir.dt.float16
AF = mybir.ActivationFunctionType
ALU = mybir.AluOpType
AX = mybir.AxisListType

# problem dims
B, S, DIM, H = 16, 16384, 32, 4
TOK = B * S                  # 262144
NCORES = 8
TPC = TOK // NCORES          # 32768 tokens per core
HK = H * DIM                 # 128

# block geometry: 10 full blocks (tpp=24) + 1 short (tpp=18) covering
# the [TPC-2304, TPC) remainder (256-token overlap, rewritten
# idempotently).
TPP = 24
BLK = 128 * TPP
T0S = [i * BLK for i in range(TPC // BLK)] + [TPC - 128 * 18]
TPPS = [TPP] * (TPC // BLK) + [18]
NBLK = len(T0S)
EPS = float(np.finfo(np.float32).eps)

# sigmoid odd-poly: sigmoid(z) ~= 0.5 + z*(c0+c1 t+c2 t^2+c3 t^3),
# t=z^2, max abs err 6e-5 on |z|<=2.4
SC0, SC1, SC2, SC3 = (2.49764353e-01, -2.02204249e-02,
                      1.63422342e-03, -7.25322973e-05)
SQRT32 = math.sqrt(32.0)

# engine split tuning (rows of tpp assigned to the named engine)
SQK_ACT = 19                 # key^2 rows on ACT (rest DVE)
SQQ_ACT = 19                 # hid^2 rows on ACT (rest DVE)
PROD_POOL = 2                # key*hid rows on Pool (rest DVE)
TREE_POOL = 4                # tree rows on Pool (rest DVE)
FIN_DVE = 8                  # final rows on DVE (rest Pool)
EVAC_DVE_SLOTS = ()              # which 2-pair evac slots go to DVE
DMA_AHEAD = 3


def _build_nc(apply_g12: bool, reps: int = 1):
    nc = bacc.Bacc(None, target_bir_lowering=False, debug=False)

    embt_d = nc.dram_tensor("embt", [NBLK * 96 * 1024], F16,
                            kind="ExternalInput")
    hid_d = nc.dram_tensor("hid", [TPC * HK], F16, kind="ExternalInput")
    wkv_d = nc.dram_tensor("wkv", [128, 480], F16, kind="ExternalInput")
    g12_d = None
    if apply_g12:
        g12_d = nc.dram_tensor("g12", [128, HK], F16, kind="ExternalInput")
    out_d = nc.dram_tensor("out", [TPC * HK], F16, kind="ExternalOutput")

    with tile.TileContext(nc) as tc, ExitStack() as ctx:
        const_p = ctx.enter_context(tc.tile_pool(name="const", bufs=1))
        hid_p = ctx.enter_context(tc.tile_pool(name="hidp", bufs=5))
        kvp_p = ctx.enter_context(
            tc.tile_pool(name="kvpp", bufs=3, space=bass.MemorySpace.PSUM))
        kv_p = ctx.enter_context(tc.tile_pool(name="kvp", bufs=3))
        sq_p = ctx.enter_context(tc.tile_pool(name="sqp", bufs=3))
        tr_p = ctx.enter_context(tc.tile_pool(name="trp", bufs=3))
        tail_p = ctx.enter_context(tc.tile_pool(name="tailp", bufs=3))
        out_p = ctx.enter_context(tc.tile_pool(name="outp", bufs=3))

        wkv_sb = const_p.tile([128, 480], F16)
        nc.sync.dma_start(wkv_sb[:], wkv_d[:])
        if apply_g12:
            g12_sb = const_p.tile([128, HK], F16)
            nc.sync.dma_start(g12_sb[:], g12_d[:])

        embt_tiles = []
        for i in range(DMA_AHEAD + 1):
            t = const_p.tile([128, 4, 2, 128], F16, name=f"embt{i}")
            nc.gpsimd.memset(t[96:128, :, :, :], 1.0)
            embt_tiles.append(t)
        NEMBT = len(embt_tiles)

        def stage_dma(b, idx):
            # input DMA issue, DMA_AHEAD blocks ahead of use.  embt rows
            # 0:96 come host-pre-transposed from DRAM; rows 96:128 stay
            # all-ones (bias rows, memset once at startup).
            t0 = T0S[b]
            tpp = TPPS[b]
            blk = 128 * tpp
            npair = tpp // 6
            embt = embt_tiles[idx % NEMBT]
            nc.sync.dma_start(
                embt[0:96, 0:npair, :, :].rearrange("p a b c -> p (a b c)"),
                embt_d[b * 96 * 1024:(b + 1) * 96 * 1024].rearrange(
                    "(p f) -> p f", p=96)[:, 0:npair * 256])
            hid_sb = hid_p.tile([128, tpp, H, DIM], F16, name="hid_sb")
            nc.sync.dma_start(
                hid_sb[:].rearrange("p a b c -> p (a b c)"),
                hid_d[t0 * HK:(t0 + blk) * HK].rearrange(
                    "(p f) -> p f", p=128))
            return embt, hid_sb

        def emit_block(b, staged, fd_rows=None):
            t0 = T0S[b]
            tpp = TPPS[b]
            blk = 128 * tpp
            npair = tpp // 6
            embt, hid_sb = staged

            # kv matmuls (fp16, 1 cyc/row) + merged ACT evac per pair
            kv_sb = kv_p.tile([128, 8, 3, 160], F16, name="kv_sb")
            for g in range(npair):
                kvp = kvp_p.tile([128, 2, 512], F32, name="kvp")
                for c2 in range(2):
                    nc.tensor.matmul(
                        kvp[:, c2, 0:480],
                        embt[:, g, c2, :],
                        wkv_sb[:, 0:480],
                        start=True, stop=True)
                nc.scalar.copy(
                    kv_sb[:, 2 * g:2 * (g + 1), :, :],
                    kvp[:, :, 0:480].rearrange("p c (j m) -> p c j m", m=160))

            key4 = kv_sb[:, 0:2 * npair, :, 0:HK].rearrange(
                "p a b (h k) -> p (a b) h k", h=H)     # [128, tpp, H, K]
            val3 = kv_sb[:, 0:2 * npair, :, HK:160].rearrange(
                "p a b k -> p (a b) k")                # [128, tpp, K]

            if apply_g12:
                hidg = sq_p.tile([128, TPP, H, DIM], F16, name="hidg")
                nc.vector.tensor_tensor(
                    hidg[:, 0:tpp], hid_sb[:],
                    g12_sb[:].rearrange("p (o h k) -> p o h k", o=1, h=H)
                    .broadcast_to([128, tpp, H, DIM]),
                    op=ALU.mult)
                hid4 = hidg[:, 0:tpp]
            else:
                hid4 = hid_sb[:]

            # squares + product into one [128, 3, tpp, H, K] tile
            sq3 = sq_p.tile([128, 3, TPP, H, DIM], F16, name="sq3")
            ka = min(SQK_ACT, tpp)
            if ka > 0:
                nc.scalar.activation(sq3[:, 0, 0:ka], key4[:, 0:ka],
                                     AF.Square)
            if ka < tpp:
                nc.vector.tensor_tensor(sq3[:, 0, ka:tpp], key4[:, ka:tpp],
                                        key4[:, ka:tpp], op=ALU.mult)
            qa = min(SQQ_ACT, tpp)
            if qa > 0:
                nc.scalar.activation(sq3[:, 1, 0:qa], hid4[:, 0:qa],
                                     AF.Square)
            if qa < tpp:
                nc.vector.tensor_tensor(sq3[:, 1, qa:tpp], hid4[:, qa:tpp],
                                        hid4[:, qa:tpp], op=ALU.mult)
            pp = min(PROD_POOL, tpp)
            if pp > 0:
                nc.gpsimd.tensor_tensor(sq3[:, 2, 0:pp], key4[:, 0:pp],
                                        hid4[:, 0:pp], op=ALU.mult)
            if pp < tpp:
                nc.vector.tensor_tensor(sq3[:, 2, pp:tpp], key4[:, pp:tpp],
                                        hid4[:, pp:tpp], op=ALU.mult)

            # 5-level pairwise add tree over k (fp16, 2x), rows split
            # DVE/Pool; L5 writes fp32 stats.
            trA = tr_p.tile([128, 3, TPP, H, 16], F16, name="trA")
            trB = tr_p.tile([128, 3, TPP, H, 8], F16, name="trB")
            stats = tr_p.tile([128, 3, TPP, H], F32, name="stats")
            tpool = min(TREE_POOL, tpp)

            def level(dst, a, b_):
                if tpool > 0:
                    nc.gpsimd.tensor_tensor(
                        dst[:, :, 0:tpool], a[:, :, 0:tpool],
                        b_[:, :, 0:tpool], op=ALU.add)
                if tpool < tpp:
                    nc.vector.tensor_tensor(
                        dst[:, :, tpool:tpp], a[:, :, tpool:tpp],
                        b_[:, :, tpool:tpp], op=ALU.add)

            s3 = sq3[:, :, 0:tpp]
            level(trA[:, :, 0:tpp], s3[:, :, :, :, 0:16],
                  s3[:, :, :, :, 16:32])
            a16 = trA[:, :, 0:tpp]
            level(trB[:, :, 0:tpp], a16[:, :, :, :, 0:8],
                  a16[:, :, :, :, 8:16])
            b8 = trB[:, :, 0:tpp]
            level(trA[:, :, 0:tpp, :, 0:4], b8[:, :, :, :, 0:4],
                  b8[:, :, :, :, 4:8])
            a4 = trA[:, :, 0:tpp, :, 0:4]
            level(trB[:, :, 0:tpp, :, 0:2], a4[:, :, :, :, 0:2],
                  a4[:, :, :, :, 2:4])
            b2 = trB[:, :, 0:tpp, :, 0:2]
            level(stats[:, :, 0:tpp].unsqueeze(4),
                  b2[:, :, :, :, 0:1], b2[:, :, :, :, 1:2])

            # ---- per-block tail ----
            FT = tpp * H
            msk = stats[:, 0, 0:tpp].rearrange("p a b -> p (a b)")
            msq = stats[:, 1, 0:tpp].rearrange("p a b -> p (a b)")
            dot = stats[:, 2, 0:tpp].rearrange("p a b -> p (a b)")

            P = tail_p.tile([128, FT], F32, name="P", tag="P")
            nc.vector.tensor_tensor(P[:], msk, msq, op=ALU.mult)
            sP = tail_p.tile([128, FT], F32, name="sP", tag="sP")
            nc.scalar.activation(sP[:], P[:], AF.Sqrt)
            w = tail_p.tile([128, FT], F32, name="w", tag="w")
            nc.vector.reciprocal(w[:], sP[:])
            ad = tail_p.tile([128, FT], F32, name="ad", tag="ad")
            nc.scalar.activation(ad[:], dot, AF.Abs, scale=SQRT32)
            t = tail_p.tile([128, FT], F32, name="t", tag="t")
            nc.vector.tensor_tensor(t[:], ad[:], w[:], op=ALU.mult)
            tr = tail_p.tile([128, FT], F32, name="tr", tag="tr")
            nc.scalar.activation(tr[:], t[:], AF.Sqrt)
            sg = tail_p.tile([128, FT], F32, name="sg", tag="sg")
            nc.scalar.activation(sg[:], dot, AF.Sign)
            h = tail_p.tile([128, FT], F32, name="h", tag="h")
            nc.vector.tensor_scalar(h[:], t[:], SC3, SC2,
                                    op0=ALU.mult, op1=ALU.add)
            h2 = tail_p.tile([128, FT], F32, name="h2", tag="h2")
            nc.vector.tensor_tensor(h2[:], h[:], t[:], op=ALU.mult)
            nc.vector.tensor_scalar(h2[:], h2[:], SC1, None, op0=ALU.add)
            nc.vector.tensor_tensor(h2[:], h2[:], t[:], op=ALU.mult)
            nc.vector.tensor_scalar(h2[:], h2[:], SC0, None, op0=ALU.add)
            nc.vector.tensor_tensor(h2[:], h2[:], tr[:], op=ALU.mult)
            nc.vector.tensor_tensor(h2[:], h2[:], sg[:], op=ALU.mult)
            gate = tail_p.tile([128, TPP, H], F16, name="gate")
            nc.vector.tensor_scalar(
                gate[:, 0:tpp].rearrange("p a b -> p (a b)"),
                h2[:], 0.5, None, op0=ALU.add)

            # ---- final gating + store ----
            # for the last blocks (fd_rows set) the final+DMA runs in
            # row halves so the first half's store overlaps the second
            # half's compute, shortening the end-of-kernel drain.
            out_sb = out_p.tile([128, TPP, H, DIM], F16, name="out_sb")
            gate_b = gate[:, 0:tpp, :].unsqueeze(3)
            val_b = val3.unsqueeze(2)
            out_f = out_d[t0 * HK:(t0 + blk) * HK].rearrange(
                "(p f) -> p f", p=128)
            if fd_rows is None:
                halves = [(0, tpp, min(FIN_DVE, tpp))]
            else:
                hm = tpp // 2
                halves = [(0, hm, hm // 2), (hm, tpp, hm // 2)]
            for r0, r1, fdh in halves:
                fd = r0 + fdh
                if fd > r0:
                    nc.vector.tensor_tensor(
                        out_sb[:, r0:fd],
                        gate_b[:, r0:fd].broadcast_to(
                            [128, fd - r0, H, DIM]),
                        val_b[:, r0:fd].broadcast_to(
                            [128, fd - r0, H, DIM]),
                        op=ALU.mult)
                if fd < r1:
                    nc.gpsimd.tensor_tensor(
                        out_sb[:, fd:r1],
                        gate_b[:, fd:r1].broadcast_to(
                            [128, r1 - fd, H, DIM]),
                        val_b[:, fd:r1].broadcast_to(
                            [128, r1 - fd, H, DIM]),
                        op=ALU.mult)
                nc.sync.dma_start(
                    out_f[:, r0 * HK:r1 * HK],
                    out_sb[:, r0:r1].rearrange("p a b c -> p (a b c)"))

        blocks = [b for _ in range(reps) for b in range(NBLK)]
        dmas = {}
        for j in range(min(DMA_AHEAD, len(blocks))):
            dmas[j] = stage_dma(blocks[j], j)
        for i, b in enumerate(blocks):
            if i + DMA_AHEAD < len(blocks):
                dmas[i + DMA_AHEAD] = stage_dma(blocks[i + DMA_AHEAD],
                                                i + DMA_AHEAD)
            # the last two blocks' finals split evenly DVE/Pool so the
            # end-of-kernel drain isn't serialized on Pool
            fd_rows = TPPS[b] // 2 if i >= len(blocks) - 3 else None
            emit_block(b, dmas.pop(i), fd_rows=fd_rows)

    nc.compile()
    return nc


def _prep_embt(emb_flat_f16):
    # embt[b, s*32+d, cc, p] = emb[t0 + p*tpp + 3*cc + s, d]; rows
    # 96:128 (the ones bias rows) live on-device, not in DRAM.
    out = np.zeros((NBLK, 96, 8, 128), dtype=np.float16)
    for b, (t0, tpp) in enumerate(zip(T0S, TPPS)):
        blk = 128 * tpp
        E = emb_flat_f16[t0:t0 + blk].reshape(128, 2 * (tpp // 6), 3, DIM)
        out[b, :, 0:2 * (tpp // 6), :] = np.transpose(
            E, (2, 3, 1, 0)).reshape(96, 2 * (tpp // 6), 128)
    return np.ascontiguousarray(out.reshape(-1))


def _prep_consts(Wv, bv, Wk, bk):
    # Wkv_cat[d, h*32+k] = Wk[h,k,d];  Wkv_cat[d, 128+v] = Wv[v,d]
    wkv_cat = np.zeros((DIM, 160), dtype=np.float32)
    wkv_cat[:, 0:HK] = np.transpose(Wk, (2, 0, 1)).reshape(DIM, HK)
    wkv_cat[:, HK:160] = Wv.T
    bias_cat = np.concatenate(
        [bk.reshape(HK).astype(np.float32), bv.astype(np.float32)])
    wkv = np.zeros((128, 480), dtype=np.float32)
    for j in range(3):
        wkv[32 * j:32 * (j + 1), 160 * j:160 * (j + 1)] = wkv_cat
    wkv[96, :] = np.tile(bias_cat, 3)
    return wkv.astype(np.float16)


_CACHE = {}


def kernel_with_results(embeddings, hidden_states, Wv, bv, Wk, bk, g1, g2,
                        **run_kwargs):
    embeddings = np.asarray(embeddings, dtype=np.float32)
    hidden_states = np.asarray(hidden_states, dtype=np.float32)
    Wv = np.asarray(Wv, dtype=np.float32)
    bv = np.asarray(bv, dtype=np.float32)
    Wk = np.asarray(Wk, dtype=np.float32)
    bk = np.asarray(bk, dtype=np.float32)
    g12 = (np.asarray(g1, np.float32) * np.asarray(g2, np.float32))
    apply_g12 = not np.all(g12 == 1.0)

    if apply_g12 not in _CACHE:
        _CACHE[apply_g12] = _build_nc(apply_g12)
    nc = _CACHE[apply_g12]

    wkv = _prep_consts(Wv, bv, Wk, bk)

    emb_flat = np.ascontiguousarray(
        embeddings.reshape(TOK, DIM).astype(np.float16))
    hid_flat = np.ascontiguousarray(
        hidden_states.reshape(TOK, HK).astype(np.float16))

    in_maps = []
    for c in range(NCORES):
        m = {
            "embt": _prep_embt(emb_flat[c * TPC:(c + 1) * TPC]),
            "hid": np.ascontiguousarray(
                hid_flat[c * TPC:(c + 1) * TPC]).reshape(-1),
            "wkv": wkv,
        }
        if apply_g12:
            m["g12"] = np.tile(
                g12.reshape(1, HK), (128, 1)).astype(np.float16)
        in_maps.append(m)

    res = run_bass_kernel_spmd(nc, in_maps, core_ids=list(range(NCORES)),
                               **run_kwargs)
    out = np.concatenate(
        [np.asarray(res.results[c]["out"]).reshape(TPC, HK)
         for c in range(NCORES)],
        axis=0)
    return out.astype(np.float32).reshape(B, S, H, DIM), res


def kernel(embeddings, hidden_states, Wv, bv, Wk, bk, g1, g2):
    out, _ = kernel_with_results(
        embeddings, hidden_states, Wv, bv, Wk, bk, g1, g2)
    return out


# revision 33
# speedup vs baseline: 1.0536x; 1.0121x over previous
"""EngramGating Trainium2 Bass kernel (fp16 pipeline, per-block tails).

Reference computation (per token t, head h, DIM=32, HC_MULT=4):
    key[t,h,:]  = emb[t,:] @ Wk[h].T + bk[h]
    nk = key * rsqrt(mean_k(key^2)+eps) * g1
    nq = hid  * rsqrt(mean_k(hid^2)+eps) * g2
    gate0[t,h] = sum_k nk*nq / sqrt(32)
    ga = sign(gate0)*sqrt(max(|gate0|,1e-6));  gate = sigmoid(ga)
    out[t,h,:] = gate[t,h] * (emb[t,:] @ Wv.T + bv)

Sharding: pure data parallel over 8 cores, contiguous token ranges.

Design (per core, tokens-on-partitions, tpp=18 tokens per partition per
block, 14 full blocks + 1 short):
 - hid arrives fp16 in DRAM; emb arrives HOST-PRE-TRANSPOSED as fp16
   stationary tiles (embt) whose rows 96:128 are kept all-ones on
   device (bias row trick); out is fp16 in DRAM (host converts back).
   fp16 is required: bf16's 8-bit mantissa gives dot errors ~0.05 that
   the sqrt at gate0~0 amplifies past the 2e-2 gate (validated
   numerically; fp16 lands at ~1e-2).
 - One K=128 fp16 matmul per chunk (1 cyc/row) against a block-diagonal
   [Wk|Wv]+bias-row constant produces key|val with biases in PSUM.
 - ACT evacuates PSUM->SBUF fp16 in one merged copy per pair.
 - Squares key^2/hid^2 split ACT(Square)/DVE(tensor_tensor, 2x fp16);
   key*hid on DVE/Pool; sum over k=32 as a 5-level pairwise add tree
   (2x fp16), rows split DVE/Pool.
 - Tail per block (ACT ops all live in one act table - no table
   switches): with S_k=sum key^2, S_q=sum hid^2, d=dot:
     t = sqrt(32)*|d| / sqrt(S_k*S_q) = |gate0|;  |z| = sqrt(t)
     gate = 0.5 + sign(d)*|z|*poly(t),  poly = minimax cubic of
     (sigmoid(z)-0.5)/z on |z| <= 32^(1/4) (Cauchy-Schwarz bound).
 - Final out = gate*val via broadcast tensor_tensor (val read straight
   from the evac tile), rows split Pool/DVE; fp16 DMA out.
"""

import math
import numpy as np
from contextlib import ExitStack

import concourse.bass as bass
import concourse.bacc as bacc
import concourse.mybir as mybir
import concourse.tile as tile
from concourse.bass_utils import run_bass_kernel_spmd

F32 = mybir.dt.float32
F16 = mybir.dt.float16
AF = mybir.ActivationFunctionType
ALU = mybir.AluOpType
AX = mybir.AxisListType

# problem dims
B, S, DIM, H = 16, 16384, 32, 4
TOK = B * S                  # 262144
NCORES = 8
TPC = TOK // NCORES          # 32768 tokens per core
HK = H * DIM                 # 128

# block geometry: 10 full blocks (tpp=24) + 1 short (tpp=18) covering
# the [TPC-2304, TPC) remainder (256-token overlap, rewritten
# idempotently).
TPP = 24
BLK = 128 * TPP
T0S = [i * BLK for i in range(TPC // BLK)] + [TPC - 128 * 18]
TPPS = [TPP] * (TPC // BLK) + [18]
NBLK = len(T0S)
EPS = float(np.finfo(np.float32).eps)

# sigmoid odd-poly: sigmoid(z) ~= 0.5 + z*(c0+c1 t+c2 t^2+c3 t^3),
# t=z^2, max abs err 6e-5 on |z|<=2.4
SC0, SC1, SC2, SC3 = (2.49764353e-01, -2.02204249e-02,
                      1.63422342e-03, -7.25322973e-05)
SQRT32 = math.sqrt(32.0)

# engine split tuning (rows of tpp assigned to the named engine)
SQK_ACT = 19                 # key^2 rows on ACT (rest DVE)
SQQ_ACT = 19                 # hid^2 rows on ACT (rest DVE)
PROD_POOL = 2                # key*hid rows on Pool (rest DVE)
TREE_POOL = 4                # tree rows on Pool (rest DVE)
FIN_DVE = 8                  # final rows on DVE (rest Pool)
EVAC_DVE_SLOTS = ()              # which 2-pair evac slots go to DVE
DMA_AHEAD = 3


def _build_nc(apply_g12: bool, reps: int = 1):
    nc = bacc.Bacc(None, target_bir_lowering=False, debug=False)

    embt_d = nc.dram_tensor("embt", [NBLK * 96 * 1024], F16,
                            kind="ExternalInput")
    hid_d = nc.dram_tensor("hid", [TPC * HK], F16, kind="ExternalInput")
    wkv_d = nc.dram_tensor("wkv", [128, 480], F16, kind="ExternalInput")
    g12_d = None
    if apply_g12:
        g12_d = nc.dram_tensor("g12", [128, HK], F16, kind="ExternalInput")
    out_d = nc.dram_tensor("out", [TPC * HK], F16, kind="ExternalOutput")

    with tile.TileContext(nc) as tc, ExitStack() as ctx:
        const_p = ctx.enter_context(tc.tile_pool(name="const", bufs=1))
        hid_p = ctx.enter_context(tc.tile_pool(name="hidp", bufs=5))
        kvp_p = ctx.enter_context(
            tc.tile_pool(name="kvpp", bufs=3, space=bass.MemorySpace.PSUM))
        kv_p = ctx.enter_context(tc.tile_pool(name="kvp", bufs=3))
        sq_p = ctx.enter_context(tc.tile_pool(name="sqp", bufs=3))
        tr_p = ctx.enter_context(tc.tile_pool(name="trp", bufs=3))
        tail_p = ctx.enter_context(tc.tile_pool(name="tailp", bufs=3))
        out_p = ctx.enter_context(tc.tile_pool(name="outp", bufs=3))

        wkv_sb = const_p.tile([128, 480], F16)
        nc.sync.dma_start(wkv_sb[:], wkv_d[:])
        if apply_g12:
            g12_sb = const_p.tile([128, HK], F16)
            nc.sync.dma_start(g12_sb[:], g12_d[:])

        embt_tiles = []
        for i in range(DMA_AHEAD + 1):
            t = const_p.tile([128, 4, 2, 128], F16, name=f"embt{i}")
            nc.gpsimd.memset(t[96:128, :, :, :], 1.0)
            embt_tiles.append(t)
        NEMBT = len(embt_tiles)

        def stage_dma(b, idx):
            # input DMA issue, DMA_AHEAD blocks ahead of use.  embt rows
            # 0:96 come host-pre-transposed from DRAM; rows 96:128 stay
            # all-ones (bias rows, memset once at startup).
            t0 = T0S[b]
            tpp = TPPS[b]
            blk = 128 * tpp
            npair = tpp // 6
            embt = embt_tiles[idx % NEMBT]
            src_f = embt_d[b * 96 * 1024:(b + 1) * 96 * 1024].rearrange(
                "(p f) -> p f", p=96)
            if idx == 0:
                # per-pair slices so the very first matmul isn't gated
                # on the whole block-0 stationary transfer
                for g in range(npair):
                    nc.sync.dma_start(
                        embt[0:96, g, :, :].rearrange("p a b -> p (a b)"),
                        src_f[:, g * 256:(g + 1) * 256])
            else:
                nc.sync.dma_start(
                    embt[0:96, 0:npair, :, :].rearrange(
                        "p a b c -> p (a b c)"),
                    src_f[:, 0:npair * 256])
            hid_sb = hid_p.tile([128, tpp, H, DIM], F16, name="hid_sb")
            nc.sync.dma_start(
                hid_sb[:].rearrange("p a b c -> p (a b c)"),
                hid_d[t0 * HK:(t0 + blk) * HK].rearrange(
                    "(p f) -> p f", p=128))
            return embt, hid_sb

        def emit_block(b, staged, fd_rows=None):
            t0 = T0S[b]
            tpp = TPPS[b]
            blk = 128 * tpp
            npair = tpp // 6
            embt, hid_sb = staged

            # kv matmuls (fp16, 1 cyc/row) + merged ACT evac per pair
            kv_sb = kv_p.tile([128, 8, 3, 160], F16, name="kv_sb")
            for g in range(npair):
                kvp = kvp_p.tile([128, 2, 512], F32, name="kvp")
                for c2 in range(2):
                    nc.tensor.matmul(
                        kvp[:, c2, 0:480],
                        embt[:, g, c2, :],
                        wkv_sb[:, 0:480],
                        start=True, stop=True)
                nc.scalar.copy(
                    kv_sb[:, 2 * g:2 * (g + 1), :, :],
                    kvp[:, :, 0:480].rearrange("p c (j m) -> p c j m", m=160))

            key4 = kv_sb[:, 0:2 * npair, :, 0:HK].rearrange(
                "p a b (h k) -> p (a b) h k", h=H)     # [128, tpp, H, K]
            val3 = kv_sb[:, 0:2 * npair, :, HK:160].rearrange(
                "p a b k -> p (a b) k")                # [128, tpp, K]

            if apply_g12:
                hidg = sq_p.tile([128, TPP, H, DIM], F16, name="hidg")
                nc.vector.tensor_tensor(
                    hidg[:, 0:tpp], hid_sb[:],
                    g12_sb[:].rearrange("p (o h k) -> p o h k", o=1, h=H)
                    .broadcast_to([128, tpp, H, DIM]),
                    op=ALU.mult)
                hid4 = hidg[:, 0:tpp]
            else:
                hid4 = hid_sb[:]

            # squares + product into one [128, 3, tpp, H, K] tile
            sq3 = sq_p.tile([128, 3, TPP, H, DIM], F16, name="sq3")
            ka = min(SQK_ACT, tpp)
            if ka > 0:
                nc.scalar.activation(sq3[:, 0, 0:ka], key4[:, 0:ka],
                                     AF.Square)
            if ka < tpp:
                nc.vector.tensor_tensor(sq3[:, 0, ka:tpp], key4[:, ka:tpp],
                                        key4[:, ka:tpp], op=ALU.mult)
            qa = min(SQQ_ACT, tpp)
            if qa > 0:
                nc.scalar.activation(sq3[:, 1, 0:qa], hid4[:, 0:qa],
                                     AF.Square)
            if qa < tpp:
                nc.vector.tensor_tensor(sq3[:, 1, qa:tpp], hid4[:, qa:tpp],
                                        hid4[:, qa:tpp], op=ALU.mult)
            pp = min(PROD_POOL, tpp)
            if pp > 0:
                nc.gpsimd.tensor_tensor(sq3[:, 2, 0:pp], key4[:, 0:pp],
                                        hid4[:, 0:pp], op=ALU.mult)
            if pp < tpp:
                nc.vector.tensor_tensor(sq3[:, 2, pp:tpp], key4[:, pp:tpp],
                                        hid4[:, pp:tpp], op=ALU.mult)

            # 5-level pairwise add tree over k (fp16, 2x), rows split
            # DVE/Pool; L5 writes fp32 stats.
            trA = tr_p.tile([128, 3, TPP, H, 16], F16, name="trA")
            trB = tr_p.tile([128, 3, TPP, H, 8], F16, name="trB")
            stats = tr_p.tile([128, 3, TPP, H], F32, name="stats")
            tpool = min(TREE_POOL, tpp)

            def level(dst, a, b_):
                if tpool > 0:
                    nc.gpsimd.tensor_tensor(
                        dst[:, :, 0:tpool], a[:, :, 0:tpool],
                        b_[:, :, 0:tpool], op=ALU.add)
                if tpool < tpp:
                    nc.vector.tensor_tensor(
                        dst[:, :, tpool:tpp], a[:, :, tpool:tpp],
                        b_[:, :, tpool:tpp], op=ALU.add)

            s3 = sq3[:, :, 0:tpp]
            level(trA[:, :, 0:tpp], s3[:, :, :, :, 0:16],
                  s3[:, :, :, :, 16:32])
            a16 = trA[:, :, 0:tpp]
            level(trB[:, :, 0:tpp], a16[:, :, :, :, 0:8],
                  a16[:, :, :, :, 8:16])
            b8 = trB[:, :, 0:tpp]
            level(trA[:, :, 0:tpp, :, 0:4], b8[:, :, :, :, 0:4],
                  b8[:, :, :, :, 4:8])
            a4 = trA[:, :, 0:tpp, :, 0:4]
            level(trB[:, :, 0:tpp, :, 0:2], a4[:, :, :, :, 0:2],
                  a4[:, :, :, :, 2:4])
            b2 = trB[:, :, 0:tpp, :, 0:2]
            level(stats[:, :, 0:tpp].unsqueeze(4),
                  b2[:, :, :, :, 0:1], b2[:, :, :, :, 1:2])

            # ---- per-block tail ----
            FT = tpp * H
            msk = stats[:, 0, 0:tpp].rearrange("p a b -> p (a b)")
            msq = stats[:, 1, 0:tpp].rearrange("p a b -> p (a b)")
            dot = stats[:, 2, 0:tpp].rearrange("p a b -> p (a b)")

            P = tail_p.tile([128, FT], F32, name="P", tag="P")
            nc.vector.tensor_tensor(P[:], msk, msq, op=ALU.mult)
            sP = tail_p.tile([128, FT], F32, name="sP", tag="sP")
            nc.scalar.activation(sP[:], P[:], AF.Sqrt)
            w = tail_p.tile([128, FT], F32, name="w", tag="w")
            nc.vector.reciprocal(w[:], sP[:])
            ad = tail_p.tile([128, FT], F32, name="ad", tag="ad")
            nc.scalar.activation(ad[:], dot, AF.Abs, scale=SQRT32)
            t = tail_p.tile([128, FT], F32, name="t", tag="t")
            nc.vector.tensor_tensor(t[:], ad[:], w[:], op=ALU.mult)
            tr = tail_p.tile([128, FT], F32, name="tr", tag="tr")
            nc.scalar.activation(tr[:], t[:], AF.Sqrt)
            sg = tail_p.tile([128, FT], F32, name="sg", tag="sg")
            nc.scalar.activation(sg[:], dot, AF.Sign)
            h = tail_p.tile([128, FT], F32, name="h", tag="h")
            nc.vector.tensor_scalar(h[:], t[:], SC3, SC2,
                                    op0=ALU.mult, op1=ALU.add)
            h2 = tail_p.tile([128, FT], F32, name="h2", tag="h2")
            nc.vector.tensor_tensor(h2[:], h[:], t[:], op=ALU.mult)
            nc.vector.tensor_scalar(h2[:], h2[:], SC1, None, op0=ALU.add)
            nc.vector.tensor_tensor(h2[:], h2[:], t[:], op=ALU.mult)
            nc.vector.tensor_scalar(h2[:], h2[:], SC0, None, op0=ALU.add)
            nc.vector.tensor_tensor(h2[:], h2[:], tr[:], op=ALU.mult)
            nc.vector.tensor_tensor(h2[:], h2[:], sg[:], op=ALU.mult)
            gate = tail_p.tile([128, TPP, H], F16, name="gate")
            nc.vector.tensor_scalar(
                gate[:, 0:tpp].rearrange("p a b -> p (a b)"),
                h2[:], 0.5, None, op0=ALU.add)

            # ---- final gating + store ----
            # for the last blocks (fd_rows set) the final+DMA runs in
            # row halves so the first half's store overlaps the second
            # half's compute, shortening the end-of-kernel drain.
            out_sb = out_p.tile([128, TPP, H, DIM], F16, name="out_sb")
            gate_b = gate[:, 0:tpp, :].unsqueeze(3)
            val_b = val3.unsqueeze(2)
            out_f = out_d[t0 * HK:(t0 + blk) * HK].rearrange(
                "(p f) -> p f", p=128)
            if fd_rows is None:
                halves = [(0, tpp, min(FIN_DVE, tpp))]
            else:
                hm = tpp // 2
                halves = [(0, hm, hm // 2), (hm, tpp, hm // 2)]
            for r0, r1, fdh in halves:
                fd = r0 + fdh
                if fd > r0:
                    nc.vector.tensor_tensor(
                        out_sb[:, r0:fd],
                        gate_b[:, r0:fd].broadcast_to(
                            [128, fd - r0, H, DIM]),
                        val_b[:, r0:fd].broadcast_to(
                            [128, fd - r0, H, DIM]),
                        op=ALU.mult)
                if fd < r1:
                    nc.gpsimd.tensor_tensor(
                        out_sb[:, fd:r1],
                        gate_b[:, fd:r1].broadcast_to(
                            [128, r1 - fd, H, DIM]),
                        val_b[:, fd:r1].broadcast_to(
                            [128, r1 - fd, H, DIM]),
                        op=ALU.mult)
                nc.sync.dma_start(
                    out_f[:, r0 * HK:r1 * HK],
                    out_sb[:, r0:r1].rearrange("p a b c -> p (a b c)"))

        blocks = [b for _ in range(reps) for b in range(NBLK)]
        dmas = {}
        for j in range(min(DMA_AHEAD, len(blocks))):
            dmas[j] = stage_dma(blocks[j], j)
        for i, b in enumerate(blocks):
            if i + DMA_AHEAD < len(blocks):
                dmas[i + DMA_AHEAD] = stage_dma(blocks[i + DMA_AHEAD],
                                                i + DMA_AHEAD)
            # the last two blocks' finals split evenly DVE/Pool so the
            # end-of-kernel drain isn't serialized on Pool
            fd_rows = TPPS[b] // 2 if i >= len(blocks) - 3 else None
            emit_block(b, dmas.pop(i), fd_rows=fd_rows)

    nc.compile()
    return nc


def _prep_embt(emb_flat_f16):
    # embt[b, s*32+d, cc, p] = emb[t0 + p*tpp + 3*cc + s, d]; rows
    # 96:128 (the ones bias rows) live on-device, not in DRAM.
    out = np.zeros((NBLK, 96, 8, 128), dtype=np.float16)
    for b, (t0, tpp) in enumerate(zip(T0S, TPPS)):
        blk = 128 * tpp
        E = emb_flat_f16[t0:t0 + blk].reshape(128, 2 * (tpp // 6), 3, DIM)
        out[b, :, 0:2 * (tpp // 6), :] = np.transpose(
            E, (2, 3, 1, 0)).reshape(96, 2 * (tpp // 6), 128)
    return np.ascontiguousarray(out.reshape(-1))


def _prep_consts(Wv, bv, Wk, bk):
    # Wkv_cat[d, h*32+k] = Wk[h,k,d];  Wkv_cat[d, 128+v] = Wv[v,d]
    wkv_cat = np.zeros((DIM, 160), dtype=np.float32)
    wkv_cat[:, 0:HK] = np.transpose(Wk, (2, 0, 1)).reshape(DIM, HK)
    wkv_cat[:, HK:160] = Wv.T
    bias_cat = np.concatenate(
        [bk.reshape(HK).astype(np.float32), bv.astype(np.float32)])
    wkv = np.zeros((128, 480), dtype=np.float32)
    for j in range(3):
        wkv[32 * j:32 * (j + 1), 160 * j:160 * (j + 1)] = wkv_cat
    wkv[96, :] = np.tile(bias_cat, 3)
    return wkv.astype(np.float16)


_CACHE = {}


def kernel_with_results(embeddings, hidden_states, Wv, bv, Wk, bk, g1, g2,
                        **run_kwargs):
    embeddings = np.asarray(embeddings, dtype=np.float32)
    hidden_states = np.asarray(hidden_states, dtype=np.float32)
    Wv = np.asarray(Wv, dtype=np.float32)
    bv = np.asarray(bv, dtype=np.float32)
    Wk = np.asarray(Wk, dtype=np.float32)
    bk = np.asarray(bk, dtype=np.float32)
    g12 = (np.asarray(g1, np.float32) * np.asarray(g2, np.float32))
    apply_g12 = not np.all(g12 == 1.0)

    if apply_g12 not in _CACHE:
        _CACHE[apply_g12] = _build_nc(apply_g12)
    nc = _CACHE[apply_g12]

    wkv = _prep_consts(Wv, bv, Wk, bk)

    emb_flat = np.ascontiguousarray(
        embeddings.reshape(TOK, DIM).astype(np.float16))
    hid_flat = np.ascontiguousarray(
        hidden_states.reshape(TOK, HK).astype(np.float16))

    in_maps = []
    for c in range(NCORES):
        m = {
            "embt": _prep_embt(emb_flat[c * TPC:(c + 1) * TPC]),
            "hid": np.ascontiguousarray(
                hid_flat[c * TPC:(c + 1) * TPC]).reshape(-1),
            "wkv": wkv,
        }
        if apply_g12:
            m["g12"] = np.tile(
                g12.reshape(1, HK), (128, 1)).astype(np.float16)
        in_maps.append(m)

    res = run_bass_kernel_spmd(nc, in_maps, core_ids=list(range(NCORES)),
                               **run_kwargs)
    out = np.concatenate(
        [np.asarray(res.results[c]["out"]).reshape(TPC, HK)
         for c in range(NCORES)],
        axis=0)
    return out.astype(np.float32).reshape(B, S, H, DIM), res


def kernel(embeddings, hidden_states, Wv, bv, Wk, bk, g1, g2):
    out, _ = kernel_with_results(
        embeddings, hidden_states, Wv, bv, Wk, bk, g1, g2)
    return out
